# revision 4
# baseline (speedup 1.0000x reference)
"""Transformer-XL multi-head attention on 8 Trainium2 NeuronCores.

Sharding: queries / relative-position tensor r are sharded over the
sequence (i) dimension - 64 query rows per core, zero collectives.

Key algebraic restructure: the reference computes r_h = r @ Wr.T
(550 GFLOP) but only ever contracts r_h with q_h and v_bias over the
small head dim, so we fold Wr into those instead:
    bd+de[b,h,i,j] = sum_e r[i,j,e] * p[b,h,i,e]
    p[b,h,i,e]     = sum_d (q_h[b,h,i,d] + v_bias[h,d]) * Wr[h*dh+d, e]
which drops total FLOPs ~9.4x.  r streams in bf16 (host pre-transposes
to put the contraction dim e on partitions).

br cancels: its contribution to the scores is constant along j, so the
softmax output is invariant to it - legitimately skipped.

Outputs: (attn_output, attn_weights) as float32, matching reference.
"""

from contextlib import ExitStack

import numpy as np
import ml_dtypes

import concourse.bass as bass
import concourse.tile as tile
from concourse import bacc, mybir, bass_utils
from concourse.bass_interp import get_hw_module
from concourse.masks import make_identity

B, S, D, H, DH = 4, 512, 1024, 16, 64
NCORES = 8
SC = S // NCORES            # 64 query rows per core
TOK = B * SC                # 256 per-core query tokens
BS = B * S                  # 2048 key/value tokens
F32 = mybir.dt.float32
BF16 = mybir.dt.bfloat16
NPBF16 = ml_dtypes.bfloat16

_CACHE = {}


# --------------------------------------------------------------------------
# device program
# --------------------------------------------------------------------------
def _emit(nc, tc, io):
    qT, kT, vT, rT = io["qT"], io["kT"], io["vT"], io["rT"]
    wqT, wkT, wvT, woT, wr = io["wqT"], io["wkT"], io["wvT"], io["woT"], io["wr"]
    ub_rep, vb_pk, bq_pk, bk_pk = io["ub_rep"], io["vb_pk"], io["bq_pk"], io["bk_pk"]
    bvv, bov = io["bvv"], io["bov"]
    out_o, out_w = io["out_o"], io["out_w"]
    bd_dram = io["bd_dram"]

    Exp = mybir.ActivationFunctionType.Exp

    with ExitStack() as root:
        persist = root.enter_context(tc.tile_pool(name="persist", bufs=1))
        q_hT = persist.tile([128, 8, TOK], F32, tag="q_hT")     # 8 KiB/p
        v_h = persist.tile([128, 16, D], BF16, tag="v_h")       # 32 KiB/p
        outT = persist.tile([128, 8, TOK], F32, tag="outT")     # 8 KiB/p
        ub_sb = persist.tile([128, 8, 64], F32, tag="ub_sb")
        vb_sb = persist.tile([128, 8], F32, tag="vb_sb")
        bq_sb = persist.tile([128, 8], F32, tag="bq_sb")
        bk_sb = persist.tile([128, 8], F32, tag="bk_sb")
        bv_sb = persist.tile([1, D], F32, tag="bv_sb")
        bo_sb = persist.tile([1, D], F32, tag="bo_sb")
        ones = persist.tile([1, 128], F32, tag="ones")
        id128 = persist.tile([128, 128], BF16, tag="id128")

        nc.sync.dma_start(out=ub_sb, in_=ub_rep)
        nc.sync.dma_start(out=vb_sb, in_=vb_pk)
        nc.sync.dma_start(out=bq_sb, in_=bq_pk)
        nc.sync.dma_start(out=bk_sb, in_=bk_pk)
        nc.sync.dma_start(out=bv_sb, in_=bvv)
        nc.sync.dma_start(out=bo_sb, in_=bov)
        nc.vector.memset(ones, 1.0)
        make_identity(nc, id128)

        # ---------------- phase Q: q_hT[d, tok] = Wq @ q^T + bq --------------
        with tc.tile_pool(name="qph", bufs=1) as qph, \
             tc.tile_pool(name="qpsum", bufs=4, space="PSUM") as qpsum:
            qTc = qph.tile([128, 8, TOK], F32, tag="qTc")
            wq_sb = qph.tile([128, 8, D], F32, tag="wq_sb")
            nc.sync.dma_start(out=qTc, in_=qT.rearrange("(c p) t -> p c t", p=128))
            nc.sync.dma_start(out=wq_sb, in_=wqT.rearrange("(c p) m -> p c m", p=128))
            for m in range(8):
                ps = qpsum.tile([128, TOK], F32, tag="qps")
                for e in range(8):
                    nc.tensor.matmul(
                        ps, wq_sb[:, e, m * 128:(m + 1) * 128], qTc[:, e, :],
                        start=(e == 0), stop=(e == 7))
                nc.vector.tensor_scalar_add(q_hT[:, m, :], ps, bq_sb[:, m:m + 1])

        # ---------------- phase V: v_h[tok, d] = v @ Wv.T + bv (bf16) --------
        with tc.tile_pool(name="vph", bufs=1) as vph, \
             tc.tile_pool(name="vpsum", bufs=4, space="PSUM") as vpsum:
            vTc = vph.tile([128, 8, BS], F32, tag="vTc")
            wv_sb = vph.tile([128, 8, D], F32, tag="wv_sb")
            nc.sync.dma_start(out=vTc, in_=vT.rearrange("(c p) t -> p c t", p=128))
            nc.sync.dma_start(out=wv_sb, in_=wvT.rearrange("(c p) m -> p c m", p=128))
            for tt in range(16):
                for mh in range(2):
                    ps = vpsum.tile([128, 512], F32, tag="vps")
                    for e in range(8):
                        nc.tensor.matmul(
                            ps, vTc[:, e, tt * 128:(tt + 1) * 128],
                            wv_sb[:, e, mh * 512:(mh + 1) * 512],
                            start=(e == 0), stop=False)
                    nc.tensor.matmul(
                        ps, ones[0:1, 0:128], bv_sb[0:1, mh * 512:(mh + 1) * 512],
                        start=False, stop=True)
                    nc.scalar.copy(out=v_h[:, tt, mh * 512:(mh + 1) * 512], in_=ps)

        # ---------------- phase P: pT[e, (b h i)] = Wr_h^T @ (q_h + vb) ------
        # ---------------- phase BD: bd[bh, j] per i -> bd_dram ---------------
        with tc.tile_pool(name="pph", bufs=1) as pph, \
             tc.tile_pool(name="wrp", bufs=2) as wrp, \
             tc.tile_pool(name="tmpp", bufs=2) as tmpp:
            pT = pph.tile([128, 8, 4096], BF16, tag="pT")       # 64 KiB/p
            with tc.tile_pool(name="ppsum", bufs=4, space="PSUM") as ppsum:
                for kk in range(8):
                    wr_t = wrp.tile([128, D], F32, tag="wr_t")
                    nc.sync.dma_start(out=wr_t, in_=wr[kk * 128:(kk + 1) * 128, :])
                    tmp = tmpp.tile([128, TOK], F32, tag="tmp")
                    for par in range(2):
                        sl = slice(par * 64, par * 64 + 64)
                        nc.vector.tensor_scalar_add(
                            tmp[sl, :], q_hT[sl, kk, :], vb_sb[sl, kk:kk + 1])
                    for par in range(2):
                        h = 2 * kk + par
                        sl = slice(par * 64, par * 64 + 64)
                        for c in range(8):
                            ps = ppsum.tile([128, TOK], F32, tag="pps")
                            nc.tensor.matmul(
                                ps, wr_t[sl, c * 128:(c + 1) * 128], tmp[sl, :],
                                start=True, stop=True)
                            dst = pT[:, c, :].rearrange(
                                "p (b hh i) -> p b hh i", b=4, hh=16)[:, :, h, :]
                            nc.vector.tensor_copy(
                                out=dst, in_=ps.rearrange("p (b i) -> p b i", b=4))

            with tc.tile_pool(name="rbp", bufs=4) as rbp, \
                 tc.tile_pool(name="stgp", bufs=3) as stgp, \
                 tc.tile_pool(name="bdpsum", bufs=3, space="PSUM") as bdpsum:
                pTr = [
                    pT[:, c, :].rearrange("p (b hh i) -> p b hh i", b=4, hh=16)
                    for c in range(8)
                ]
                for t in range(SC // 2):
                    rb = []
                    for par in range(2):
                        rbt = rbp.tile([128, 8, 512], BF16, tag="rb")
                        nc.sync.dma_start(
                            out=rbt,
                            in_=rT[2 * t + par].rearrange("c e j -> e c j"))
                        rb.append(rbt)
                    ps = bdpsum.tile([128, 512], F32, tag="bdps")
                    for c in range(8):
                        nc.tensor.matmul(
                            ps[0:64, :], pTr[c][:, :, :, 2 * t], rb[0][:, c, :],
                            start=(c == 0), stop=(c == 7), tile_position=(0, 0))
                        nc.tensor.matmul(
                            ps[64:128, :], pTr[c][:, :, :, 2 * t + 1], rb[1][:, c, :],
                            start=(c == 0), stop=(c == 7), tile_position=(0, 64))
                    stg = stgp.tile([128, 512], BF16, tag="stg")
                    nc.vector.tensor_copy(out=stg, in_=ps)
                    # bd_dram[i, bh, j]; partitions (par,bh) flatten to (i bh)
                    nc.sync.dma_start(
                        out=bd_dram[2 * t:2 * t + 2].rearrange("i bh j -> (i bh) j"),
                        in_=stg)

        # ---------------- phases K / PAIRS / O -------------------------------
        with tc.tile_pool(name="khp", bufs=1) as khp:
            k_hT = khp.tile([128, 8, BS], F32, tag="k_hT")      # 64 KiB/p
            wo_sb = khp.tile([128, 8, D], F32, tag="wo_sb")     # 32 KiB/p
            nc.sync.dma_start(out=wo_sb, in_=woT.rearrange("(c p) m -> p c m", p=128))

            # ------------- phase K: k_hT[d, tok] = Wk @ k^T + bk -------------
            with tc.tile_pool(name="kwp", bufs=1) as kwp, \
                 tc.tile_pool(name="ktq", bufs=3) as ktq, \
                 tc.tile_pool(name="kpsum", bufs=8, space="PSUM") as kpsum:
                wk_sb = kwp.tile([128, 8, D], F32, tag="wk_sb")
                nc.sync.dma_start(out=wk_sb,
                                  in_=wkT.rearrange("(c p) m -> p c m", p=128))
                for tq in range(4):
                    kps = [kpsum.tile([128, 512], F32, tag="kps",
                                      name=f"kps_{tq}_{m}")
                           for m in range(8)]
                    for e in range(8):
                        kte = ktq.tile([128, 512], F32, tag="kte")
                        nc.sync.dma_start(
                            out=kte,
                            in_=kT[e * 128:(e + 1) * 128, tq * 512:(tq + 1) * 512])
                        for m in range(8):
                            nc.tensor.matmul(
                                kps[m], wk_sb[:, e, m * 128:(m + 1) * 128], kte,
                                start=(e == 0), stop=(e == 7))
                    for m in range(8):
                        nc.vector.tensor_scalar_add(
                            k_hT[:, m, tq * 512:(tq + 1) * 512], kps[m],
                            bk_sb[:, m:m + 1])

            # ------------- phase PAIRS: scores, softmax, attn@v --------------
            with tc.tile_pool(name="bdbp", bufs=4) as bdbp, \
                 tc.tile_pool(name="scp", bufs=3) as scp, \
                 tc.tile_pool(name="expp", bufs=3) as expp, \
                 tc.tile_pool(name="redp", bufs=6) as redp, \
                 tc.tile_pool(name="abp", bufs=2) as abp, \
                 tc.tile_pool(name="atp", bufs=2) as atp, \
                 tc.tile_pool(name="acpsum", bufs=2, space="PSUM") as acpsum, \
                 tc.tile_pool(name="tppsum", bufs=3, space="PSUM") as tppsum, \
                 tc.tile_pool(name="avpsum", bufs=2, space="PSUM") as avpsum:
                for pb in range(32):
                    b, kk = pb // 8, pb % 8
                    h0, h1 = 2 * kk, 2 * kk + 1
                    bdb = bdbp.tile([128, 512], BF16, tag="bdb")
                    # src iterates (bh, i, j): partition p = bh*64 + i
                    bd_src = bass.AP(
                        tensor=bd_dram.tensor, offset=2 * pb * S,
                        ap=[[S, 2], [64 * S, SC], [1, S]])
                    nc.sync.dma_start(out=bdb, in_=bd_src)
                    ps = acpsum.tile([128, 512], F32, tag="acps")
                    for par, h in ((0, h0), (1, h1)):
                        sl = slice(par * 64, par * 64 + 64)
                        nc.tensor.matmul(
                            ps[sl, :],
                            q_hT[sl, kk, b * 64:(b + 1) * 64],
                            k_hT[sl, kk, b * 512:(b + 1) * 512],
                            start=True, stop=False,
                            tile_position=(par * 64, par * 64))
                        nc.tensor.matmul(
                            ps[sl, :],
                            ub_sb[sl, kk, :],
                            k_hT[sl, kk, b * 512:(b + 1) * 512],
                            start=False, stop=True,
                            tile_position=(par * 64, par * 64))
                    sc = scp.tile([128, 512], F32, tag="sc")
                    nc.vector.tensor_add(sc, ps, bdb)
                    ex = expp.tile([128, 512], F32, tag="ex")
                    sums = redp.tile([128, 1], F32, tag="sums")
                    nc.scalar.activation(out=ex, in_=sc, func=Exp, scale=0.125,
                                         accum_out=sums)
                    rec = redp.tile([128, 1], F32, tag="rec")
                    nc.vector.reciprocal(rec, sums)
                    nc.vector.tensor_scalar_mul(ex, ex, rec)
                    nc.sync.dma_start(
                        out=out_w[b, h0:h0 + 2].rearrange("h i j -> (h i) j"),
                        in_=ex)
                    ab = abp.tile([128, 512], BF16, tag="ab")
                    nc.vector.tensor_copy(out=ab, in_=ex)
                    at = atp.tile([128, 2, 4, 64], BF16, tag="at")
                    for par in range(2):
                        sl = slice(par * 64, par * 64 + 64)
                        for jc in range(4):
                            tp = tppsum.tile([128, 64], BF16, tag="tp")
                            nc.tensor.transpose(
                                tp, ab[sl, jc * 128:(jc + 1) * 128], id128[sl, sl])
                            nc.vector.tensor_copy(out=at[:, par, jc, :], in_=tp)
                    avp = avpsum.tile([128, 64], F32, tag="avp")
                    for jc in range(4):
                        nc.tensor.matmul(
                            avp[0:64, :],
                            v_h[:, b * 4 + jc, h0 * 64:(h0 + 1) * 64],
                            at[:, 0, jc, :],
                            start=(jc == 0), stop=(jc == 3), tile_position=(0, 0))
                        nc.tensor.matmul(
                            avp[64:128, :],
                            v_h[:, b * 4 + jc, h1 * 64:(h1 + 1) * 64],
                            at[:, 1, jc, :],
                            start=(jc == 0), stop=(jc == 3), tile_position=(0, 64))
                    nc.vector.tensor_copy(out=outT[:, kk, b * 64:(b + 1) * 64],
                                          in_=avp)

            # ------------- phase O: attn_out = outT^T @ Wo^T + bo ------------
            with tc.tile_pool(name="oph", bufs=2) as oph, \
                 tc.tile_pool(name="opsum", bufs=2, space="PSUM") as opsum:
                for tch in range(2):
                    for mh in range(2):
                        ps = opsum.tile([128, 512], F32, tag="ops")
                        for c in range(8):
                            nc.tensor.matmul(
                                ps, outT[:, c, tch * 128:(tch + 1) * 128],
                                wo_sb[:, c, mh * 512:(mh + 1) * 512],
                                start=(c == 0), stop=False)
                        nc.tensor.matmul(
                            ps, ones[0:1, 0:128],
                            bo_sb[0:1, mh * 512:(mh + 1) * 512],
                            start=False, stop=True)
                        osb = oph.tile([128, 512], F32, tag="osb")
                        nc.scalar.copy(out=osb, in_=ps)
                        nc.sync.dma_start(
                            out=out_o.rearrange("b s d -> (b s) d")[
                                tch * 128:(tch + 1) * 128,
                                mh * 512:(mh + 1) * 512],
                            in_=osb)


def _build():
    if "nc" in _CACHE:
        return _CACHE["nc"]
    nc = bacc.Bacc("TRN2", target_bir_lowering=False, debug=False,
                   num_devices=NCORES)
    io = {}
    io["qT"] = nc.dram_tensor("qT", (D, TOK), F32, kind="ExternalInput").ap()
    io["kT"] = nc.dram_tensor("kT", (D, BS), F32, kind="ExternalInput").ap()
    io["vT"] = nc.dram_tensor("vT", (D, BS), F32, kind="ExternalInput").ap()
    io["rT"] = nc.dram_tensor("rT", (SC, 8, 128, S), BF16,
                              kind="ExternalInput").ap()
    io["wqT"] = nc.dram_tensor("wqT", (D, D), F32, kind="ExternalInput").ap()
    io["wkT"] = nc.dram_tensor("wkT", (D, D), F32, kind="ExternalInput").ap()
    io["wvT"] = nc.dram_tensor("wvT", (D, D), F32, kind="ExternalInput").ap()
    io["woT"] = nc.dram_tensor("woT", (D, D), F32, kind="ExternalInput").ap()
    io["wr"] = nc.dram_tensor("wr", (D, D), F32, kind="ExternalInput").ap()
    io["ub_rep"] = nc.dram_tensor("ub_rep", (128, 8, 64), F32,
                                  kind="ExternalInput").ap()
    io["vb_pk"] = nc.dram_tensor("vb_pk", (128, 8), F32,
                                 kind="ExternalInput").ap()
    io["bq_pk"] = nc.dram_tensor("bq_pk", (128, 8), F32,
                                 kind="ExternalInput").ap()
    io["bk_pk"] = nc.dram_tensor("bk_pk", (128, 8), F32,
                                 kind="ExternalInput").ap()
    io["bvv"] = nc.dram_tensor("bvv", (1, D), F32, kind="ExternalInput").ap()
    io["bov"] = nc.dram_tensor("bov", (1, D), F32, kind="ExternalInput").ap()
    io["out_o"] = nc.dram_tensor("out_o", (B, SC, D), F32,
                                 kind="ExternalOutput").ap()
    io["out_w"] = nc.dram_tensor("out_w", (B, H, SC, S), F32,
                                 kind="ExternalOutput").ap()
    io["bd_dram"] = nc.dram_tensor("bd_dram", (SC, 64, S), BF16,
                                   kind="Internal").ap()
    with tile.TileContext(nc) as tc:
        _emit(nc, tc, io)
    nc.compile()
    nc.m = get_hw_module(nc.m)
    _CACHE["nc"] = nc
    return nc


# --------------------------------------------------------------------------
# host side: shard + layout prep, run, gather
# --------------------------------------------------------------------------
def _prep_core_inputs(inputs):
    f = lambda name: np.asarray(inputs[name], np.float32)
    q, k, v, r = f("q"), f("k"), f("v"), np.asarray(inputs["r"], np.float32)
    Wq, Wk, Wv, Wr, Wo = f("Wq"), f("Wk"), f("Wv"), f("Wr"), f("Wo")
    u_bias, v_bias = f("u_bias"), f("v_bias")
    bq, bk, bv, bo = f("bq"), f("bk"), f("bv"), f("bo")

    kT = np.ascontiguousarray(k.reshape(BS, D).T)           # (D, BS)
    vT = np.ascontiguousarray(v.reshape(BS, D).T)
    shared = dict(
        kT=kT, vT=vT,
        wqT=np.ascontiguousarray(Wq.T), wkT=np.ascontiguousarray(Wk.T),
        wvT=np.ascontiguousarray(Wv.T), woT=np.ascontiguousarray(Wo.T),
        wr=np.ascontiguousarray(Wr),
        ub_rep=np.ascontiguousarray(np.broadcast_to(
            u_bias.reshape(8, 2, DH).transpose(1, 2, 0).reshape(128, 8, 1),
            (128, 8, 64))),
        vb_pk=np.ascontiguousarray(
            v_bias.reshape(8, 2, DH).transpose(1, 2, 0).reshape(128, 8)),
        bq_pk=np.ascontiguousarray(bq.reshape(8, 128).T),
        bk_pk=np.ascontiguousarray(bk.reshape(8, 128).T),
        bvv=bv.reshape(1, D).copy(),
        bov=bo.reshape(1, D).copy(),
    )

    in_maps = []
    for c in range(NCORES):
        i0 = c * SC
        m = dict(shared)
        m["qT"] = np.ascontiguousarray(q[:, i0:i0 + SC, :].reshape(TOK, D).T)
        m["rT"] = np.ascontiguousarray(
            r[i0:i0 + SC].transpose(0, 2, 1)).astype(NPBF16).reshape(
                SC, 8, 128, S)
        in_maps.append(m)
    return in_maps


def kernel(**inputs):
    nc = _build()
    in_maps = _prep_core_inputs(inputs)
    res = bass_utils.run_bass_kernel_spmd(nc, in_maps,
                                          core_ids=list(range(NCORES)))
    attn_output = np.empty((B, S, D), np.float32)
    attn_weights = np.empty((B, H, S, S), np.float32)
    for c in range(NCORES):
        i0 = c * SC
        attn_output[:, i0:i0 + SC, :] = res.results[c]["out_o"]
        attn_weights[:, :, i0:i0 + SC, :] = res.results[c]["out_w"]
    return attn_output, attn_weights
